# revision 9
# baseline (speedup 1.0000x reference)
"""Expert-choice MoE FFN (B=2, S=2048, D=1024, E=16, k=256) on 8 trn2 cores.

Sharding: 8 cores = 2 batch shards x 4 expert-group shards (4 experts each).
Each core gets its batch's x (twice: transposed for the gate matmul, row-major
in DRAM for gathers) and its 4 experts' W1/W2/b1; b2 is replicated. The core
computes a partial y for its batch (scatter-add of its experts only); the host
sums the 4 group-partials per batch.

Per core:
  - logits^T (4, S) = gate_g^T @ x_b^T          (PE, exact fp32 for routing)
  - softmax stats over tokens                   (DVE reduce + ACT exp/accum)
  - top-256 per expert row in three stages:
      L0: per 256-token chunk (32 partitions) extract top-64 candidates
          (max8 + match_replace rounds; 64 >> expected per-chunk share 32)
      finish: flat top-256 of the 512 candidates per row (values only)
      re-find: recover token indices by max_index value-matching against the
          pristine chunked logits, then fold chunk offsets with a tiny matmul
  - per expert: dma_gather token rows, pre-scale by gate prob g (valid since
    g>0 commutes with relu), PE transpose, 2-layer FFN in fp32r with biases
    applied as K=1 fp32 matmul rows (b1*g, b2*g), dma_scatter_add into y.
"""

import os
import sys

sys.path.insert(0, "/opt/trn_rl_repo")

import numpy as np

B, S, D, E = 2, 2048, 1024, 16
NCORES = 8
NG = 4           # expert-group shards
EG = E // NG     # experts per core
K = 256          # top-k
PD = 128
KD = D // PD     # contraction chunks
TB = K // PD     # token blocks of 128
NEG = -3.0e38

NCH = 8          # token chunks per row for topk L0
CH = S // NCH    # 256 tokens per chunk
R0 = 56          # candidates kept per chunk (measured max share is 47)
NCAND = NCH * R0  # 512 candidates per row

_MM_FP32 = os.environ.get("KERNEL_MM_FP32", "0") == "1"

_cache = {}


def _build_nc(repeats=1):
    import concourse.bacc as bacc
    import concourse.mybir as mybir
    import concourse.tile as tile

    dt = mybir.dt
    Act = mybir.ActivationFunctionType
    RMM = dt.float32 if _MM_FP32 else dt.float32r

    nc = bacc.Bacc("TRN2", target_bir_lowering=False, debug=False, num_devices=NCORES)

    xT_d = nc.dram_tensor("xT", [D, S], dt.float32, kind="ExternalInput")
    xrow_d = nc.dram_tensor("xrows", [S, D], dt.float32, kind="ExternalInput")
    gate_d = nc.dram_tensor("gateg", [D, EG], dt.float32, kind="ExternalInput")
    w1_d = nc.dram_tensor("w1g", [EG, D, D], dt.float32, kind="ExternalInput")
    b1_d = nc.dram_tensor("b1g", [1, EG * D], dt.float32, kind="ExternalInput")
    w2_d = nc.dram_tensor("w2g", [EG, D, D], dt.float32, kind="ExternalInput")
    b2_d = nc.dram_tensor("b2v", [1, D], dt.float32, kind="ExternalInput")
    id_d = nc.dram_tensor("ident", [PD, PD], dt.float32, kind="ExternalInput")
    # chunked-topk partition layout: p = NCH*... -> p = 4*c + r (row r, chunk c)
    # choff[p] = CH * (p // EG); smat[p, r] = 1.0 if p % EG == r else 0
    choff_d = nc.dram_tensor("choff", [EG * NCH, 1], dt.float32, kind="ExternalInput")
    smat_d = nc.dram_tensor("smat", [EG * NCH, EG], dt.float32, kind="ExternalInput")
    y_d = nc.dram_tensor("y", [S, D], dt.float32, kind="ExternalOutput")
    # scratch DRAM for cross-partition reshapes
    ldr_d = nc.dram_tensor("ldr", [EG, S], dt.float32)
    cdr_d = nc.dram_tensor("cdr", [EG * NCH, R0], dt.float32)

    NP0 = EG * NCH  # 32 partitions used by the chunked topk stages

    with tile.TileContext(nc) as tc:
        with tc.tile_pool(name="persist", bufs=1) as pp:
            ident_sb = pp.tile([PD, PD], dt.float32, tag="ident")
            nc.sync.dma_start(ident_sb[:], id_d[:])
            b2_sb = pp.tile([1, D], dt.float32, tag="b2")
            nc.sync.dma_start(b2_sb[:], b2_d[:])
            b1_sb = pp.tile([1, EG * D], dt.float32, tag="b1")
            nc.sync.dma_start(b1_sb[:], b1_d[:])
            choff_sb = pp.tile([NP0, 1], dt.float32, tag="choff")
            nc.sync.dma_start(choff_sb[:], choff_d[:])
            smat_sb = pp.tile([NP0, EG], dt.float32, tag="smat")
            nc.sync.dma_start(smat_sb[:], smat_d[:])

            lchunk = pp.tile([NP0, CH], dt.float32, tag="lchunk")   # pristine
            lwork = pp.tile([NP0, CH], dt.float32, tag="lwork")     # destroyed
            cand = pp.tile([NP0, R0], dt.float32, tag="cand")
            candflat = pp.tile([EG, NCAND], dt.float32, tag="candflat")
            gv = pp.tile([EG, K], dt.float32, tag="gv")
            gvr = pp.tile([NP0, K], dt.float32, tag="gvr")
            posall = pp.tile([NP0, K], dt.uint16, tag="posall")
            posf = pp.tile([NP0, K], dt.float32, tag="posf")
            posmask = pp.tile([NP0, K], dt.float32, tag="posmask")
            post = pp.tile([NP0, K], dt.float32, tag="post")
            gi = pp.tile([EG, K], dt.uint16, tag="gi")
            sumexp = pp.tile([EG, 1], dt.float32, tag="sumexp")
            recip = pp.tile([EG, 1], dt.float32, tag="recip")
            gexp = pp.tile([EG, K], dt.float32, tag="gexp")
            g_all = pp.tile([EG, K], dt.float32, tag="g_all")

            # (no y zero-fill: both run_bass_kernel_spmd paths pre-zero
            # ExternalOutput buffers before the kernel runs)

            for rep in range(repeats):
              with (
                tc.tile_pool(name=f"w1p{rep}", bufs=3) as w1p,
                tc.tile_pool(name=f"w2p{rep}", bufs=2) as w2p,
                tc.tile_pool(name=f"xselp{rep}", bufs=3) as xsp,
                tc.tile_pool(name=f"xsTp{rep}", bufs=8) as xstp,
                tc.tile_pool(name=f"hp{rep}", bufs=8) as hp,
                tc.tile_pool(name=f"outp{rep}", bufs=2) as outp,
                tc.tile_pool(name=f"idxp{rep}", bufs=4) as idxp,
              ):
                # W loads first, on the ACT DMA queue: no data deps and a
                # dedicated queue, so they stream from t=0 and overlap the
                # whole routing phase instead of trickling through phase D.
                w1_tiles, w2_tiles = [], []
                for e in range(EG):
                    wh = []
                    for half in range(2):
                        t = w1p.tile(
                            [PD, KD // 2, D], RMM, tag="w1",
                            name=f"w1_{rep}_{e}_{half}",
                        )
                        nc.scalar.dma_start(
                            t[:],
                            w1_d[
                                e, half * (D // 2):(half + 1) * (D // 2), :
                            ].bitcast(RMM).rearrange("(kk p) d -> p kk d", p=PD),
                        )
                        wh.append(t)
                    w1_tiles.append(wh)
                    wh = []
                    for half in range(2):
                        t = w2p.tile(
                            [PD, KD // 2, D], RMM, tag="w2",
                            name=f"w2_{rep}_{e}_{half}",
                        )
                        nc.scalar.dma_start(
                            t[:],
                            w2_d[
                                e, half * (D // 2):(half + 1) * (D // 2), :
                            ].bitcast(RMM).rearrange("(kk p) d -> p kk d", p=PD),
                        )
                        wh.append(t)
                    w2_tiles.append(wh)
                # ---- Phase A: logits^T (EG, S) = gate_g^T @ x_b^T ----
                # k-outer so the first matmul fires after one 1MB x chunk
                with (
                    tc.tile_pool(name=f"xTp{rep}", bufs=2) as xTp,
                    tc.tile_pool(name=f"lsb{rep}", bufs=1) as lsbp,
                    tc.tile_pool(name=f"gatep{rep}", bufs=1) as gp,
                    tc.tile_pool(name=f"lpsum{rep}", bufs=4, space="PSUM") as lp,
                    tc.tile_pool(name=f"wups{rep}", bufs=1, space="PSUM") as wup,
                ):
                    # PE warmup: ~3.5us of back-to-back identity matmuls so
                    # the clock gate opens before the fp32 logits matmuls
                    # (fills the initial DMA wait with PE activity).
                    wtile = wup.tile([PD, PD], dt.float32, tag="warm")
                    for _ in range(32):
                        nc.tensor.matmul(
                            wtile[:], ident_sb[:], ident_sb[:],
                            start=True, stop=True,
                        )
                    gate_sb = gp.tile([PD, KD, EG], dt.float32, tag="gate")
                    nc.sync.dma_start(
                        gate_sb[:], gate_d[:].rearrange("(k p) e -> p k e", p=PD)
                    )
                    NL = S // 512
                    lps = [
                        lp.tile([EG, 512], dt.float32, tag="lps", name=f"lps{rep}_{n}")
                        for n in range(NL)
                    ]
                    for k in range(KD):
                        xt = xTp.tile([PD, S], dt.float32, tag="xT")
                        nc.sync.dma_start(xt[:], xT_d[k * PD:(k + 1) * PD, :])
                        for n in range(NL):
                            nc.tensor.matmul(
                                lps[n][:],
                                gate_sb[:, k, :],
                                xt[:, n * 512:(n + 1) * 512],
                                start=(k == 0),
                                stop=(k == KD - 1),
                            )
                    # stage logits to SBUF (DMA cannot read PSUM), then
                    # roundtrip via DRAM into the chunked layout
                    # (partition EG*c + r holds logits[r, CH*c:CH*(c+1)])
                    logits_sb = lsbp.tile(
                        [EG, S], dt.float32, tag="logsb", name=f"logsb{rep}"
                    )
                    for n in range(NL):
                        nc.vector.tensor_copy(
                            logits_sb[:, n * 512:(n + 1) * 512], lps[n][:]
                        )
                    nc.gpsimd.dma_start(ldr_d[:], logits_sb[:])
                nc.gpsimd.dma_start(
                    lchunk[:],
                    ldr_d[:].rearrange("r (c t) -> c r t", c=NCH),
                )
                nc.vector.tensor_copy(lwork[:], lchunk[:])

                # softmax stats in chunk layout (overlaps the L0 extraction).
                # Constant stability shift 0: logits are ~N(0,1), exp cannot
                # overflow fp32, and any constant cancels in the ratio.
                with tc.tile_pool(name=f"scratchp{rep}", bufs=1) as sp:
                    esc = sp.tile([NP0, CH], dt.float32, tag="esc")
                    acc32 = sp.tile([NP0, 1], dt.float32, tag="acc32")
                    nc.scalar.activation(
                        esc[:],
                        lchunk[:],
                        Act.Exp,
                        bias=0.0,
                        scale=1.0,
                        accum_out=acc32[:, 0:1],
                    )
                    with tc.tile_pool(
                        name=f"sepsum{rep}", bufs=1, space="PSUM"
                    ) as sep:
                        seps = sep.tile([EG, 1], dt.float32, tag="seps")
                        nc.tensor.matmul(
                            seps[:], smat_sb[:], acc32[:], start=True, stop=True
                        )
                        nc.vector.tensor_copy(sumexp[:], seps[:])
                nc.vector.reciprocal(recip[:], sumexp[:])

                # ---- Phase B: top-256 per row ----
                # L0: top-R0 of each chunk
                for r in range(R0 // 8):
                    cv = cand[:, 8 * r:8 * r + 8]
                    nc.vector.max(cv, lwork[:])
                    nc.vector.match_replace(lwork[:], cv, lwork[:], NEG)
                # merge candidates into one row per expert (via DRAM)
                nc.gpsimd.dma_start(cdr_d[:], cand[:])
                nc.gpsimd.dma_start(
                    candflat[:],
                    cdr_d[:].rearrange("(c r) j -> r c j", c=NCH),
                )
                # finish: top-K values of the candidates (sorted desc)
                for r in range(K // 8):
                    mv = gv[:, 8 * r:8 * r + 8]
                    nc.vector.max(mv, candflat[:])
                    nc.vector.match_replace(candflat[:], mv, candflat[:], NEG)
                # re-find token indices: replicate gv to each chunk block
                # (log-doubling), then value-match per chunk
                nc.gpsimd.dma_start(gvr[0:EG, :], gv[:])
                w = EG
                while w < EG * NCH:
                    nc.gpsimd.dma_start(gvr[w:2 * w, :], gvr[0:w, :])
                    w *= 2
                for r in range(K // 8):
                    nc.vector.max_index(
                        posall[:, 8 * r:8 * r + 8], gvr[:, 8 * r:8 * r + 8], lchunk[:]
                    )
                nc.vector.tensor_copy(posf[:], posall[:])
                nc.vector.tensor_single_scalar(
                    posmask[:], posf[:], 60000.0, mybir.AluOpType.is_lt
                )
                nc.vector.tensor_scalar_add(post[:], posf[:], choff_sb[:, 0:1])
                nc.vector.tensor_mul(post[:], post[:], posmask[:])
                with tc.tile_pool(name=f"gpsum{rep}", bufs=1, space="PSUM") as gpp:
                    gps = gpp.tile([EG, K], dt.float32, tag="gps")
                    nc.tensor.matmul(gps[:], smat_sb[:], post[:], start=True, stop=True)
                    nc.vector.tensor_copy(gi[:], gps[:])

                # ---- Phase C: gate probabilities of the selected tokens ----
                nc.scalar.activation(
                    gexp[:], gv[:], Act.Exp, bias=0.0, scale=1.0
                )
                nc.vector.tensor_scalar_mul(g_all[:], gexp[:], recip[:, 0:1])

                # ---- Phase D: per-expert gather -> FFN -> scatter-add ----
                with (
                    tc.tile_pool(name=f"pst{rep}", bufs=2, space="PSUM") as pst,
                    tc.tile_pool(name=f"ps1{rep}", bufs=3, space="PSUM") as ps1,
                    tc.tile_pool(name=f"ps2{rep}", bufs=2, space="PSUM") as ps2,
                ):
                    # Index/gate prep + gathers for ALL experts up front, so
                    # the in-order Pool queue never parks a gather behind a
                    # scatter of an earlier expert.
                    idx_wrs, g_rows, g_cols, x_sels = [], [], [], []
                    for e in range(EG):
                        # The DMA gather/scatter unwrap idxs as [16, K//16]
                        # with list position i at [i%16, i//16]. Writing the
                        # row PLAIN into (16,16) permutes the list order by
                        # the 16x16 transpose pi(i) = 16*(i%16) + i//16 —
                        # fine as long as g uses the same permuted order.
                        # Replicate to all 8 groups by log-doubling.
                        idx_wr = idxp.tile(
                            [PD, K // 16], dt.uint16, tag="idxwr",
                            name=f"idxwr_{rep}_{e}", bufs=4,
                        )
                        nc.sync.dma_start(idx_wr[0:16, :], gi[e:e + 1, :])
                        w = 16
                        while w < PD:
                            nc.sync.dma_start(idx_wr[w:2 * w, :], idx_wr[0:w, :])
                            w *= 2
                        # g in permuted stream order (bias-row operand, base 0)
                        g_tmp0 = idxp.tile(
                            [1, K], dt.float32, tag="gtmp0",
                            name=f"gtmp0_{rep}_{e}", bufs=2,
                        )
                        nc.sync.dma_start(g_tmp0[:], g_all[e:e + 1, :])
                        g_row = idxp.tile(
                            [1, K], dt.float32, tag="grow",
                            name=f"grow_{rep}_{e}", bufs=4,
                        )
                        nc.vector.tensor_copy(
                            g_row[:], g_tmp0[:].rearrange("a (s p) -> a p s", p=16)
                        )
                        # (128,2) per-partition scale: g_col[p,c]=g_row[128c+p]
                        g_colrow = idxp.tile(
                            [1, K], dt.float32, tag="gcolrow",
                            name=f"gcolrow_{rep}_{e}", bufs=2,
                        )
                        nc.vector.tensor_copy(
                            g_colrow[:].rearrange("a (p c) -> a p c", p=PD),
                            g_row[:].rearrange("a (c p) -> a p c", p=PD),
                        )
                        g_col = idxp.tile(
                            [PD, TB], dt.float32, tag="gcol",
                            name=f"gcol_{rep}_{e}", bufs=4,
                        )
                        nc.sync.dma_start(g_col[:], g_colrow[:])

                        x_sel = xsp.tile(
                            [PD, TB, D], dt.float32, tag="xsel",
                            name=f"xsel_{rep}_{e}",
                        )
                        nc.gpsimd.dma_gather(
                            x_sel[:], xrow_d[:], idx_wr[:].bitcast(dt.int16), K, K, D
                        )
                        idx_wrs.append(idx_wr)
                        g_rows.append(g_row)
                        g_cols.append(g_col)
                        x_sels.append(x_sel)

                    for e in range(EG):
                        idx_wr = idx_wrs[e]
                        g_row = g_rows[e]
                        g_col = g_cols[e]
                        x_sel = x_sels[e]
                        w1_at = lambda k: w1_tiles[e][k // 4][:, k % 4, :]  # noqa: E731, B023
                        w2_at = lambda k: w2_tiles[e][k // 4][:, k % 4, :]  # noqa: E731, B023
                        for c in range(TB):
                            nc.scalar.activation(
                                x_sel[:, c, :],
                                x_sel[:, c, :],
                                Act.Copy,
                                bias=0.0,
                                scale=g_col[:, c:c + 1],
                            )
                        # transpose to (d, token); ACT copies round to fp32r
                        xsT = [
                            xstp.tile([PD, K], RMM, tag="xsT", name=f"xsT_{rep}_{e}_{j}")
                            for j in range(KD)
                        ]
                        for j in range(KD):
                            for c in range(TB):
                                tp = pst.tile([PD, PD], dt.float32, tag="trp")
                                nc.tensor.transpose(
                                    tp[:], x_sel[:, c, j * PD:(j + 1) * PD], ident_sb[:]
                                )
                                nc.vector.tensor_copy(
                                    xsT[j][:, c * PD:(c + 1) * PD], tp[:]
                                )

                        h_tiles = []
                        for m in range(KD):
                            ph = ps1.tile([PD, K], dt.float32, tag="ps1")
                            for k in range(KD):
                                nc.tensor.matmul(
                                    ph[:],
                                    w1_at(k)[:, m * PD:(m + 1) * PD],
                                    xsT[k][:],
                                    start=(k == 0),
                                    stop=False,
                                )
                            # + g * b1  (K=1 fp32 matmul row)
                            nc.tensor.matmul(
                                ph[:],
                                b1_sb[0:1, e * D + m * PD:e * D + (m + 1) * PD],
                                g_row[:],
                                start=False,
                                stop=True,
                            )
                            ht = hp.tile([PD, K], RMM, tag="h")
                            nc.scalar.activation(
                                ht[:], ph[:], Act.Relu, bias=0.0, scale=1.0
                            )
                            h_tiles.append(ht)

                        out_sb = outp.tile([PD, TB, D], dt.float32, tag="outsb")
                        for m2 in range(TB):
                            for n in range(2):
                                po = ps2.tile([PD, 512], dt.float32, tag="ps2")
                                for k in range(KD):
                                    nc.tensor.matmul(
                                        po[:],
                                        h_tiles[k][:, m2 * PD:(m2 + 1) * PD],
                                        w2_at(k)[:, n * 512:(n + 1) * 512],
                                        start=(k == 0),
                                        stop=False,
                                    )
                                # + g * b2  (K=1 fp32 matmul row)
                                nc.tensor.matmul(
                                    po[:],
                                    g_row[0:1, m2 * PD:(m2 + 1) * PD],
                                    b2_sb[:, n * 512:(n + 1) * 512],
                                    start=False,
                                    stop=True,
                                )
                                nc.vector.tensor_copy(
                                    out_sb[:, m2, n * 512:(n + 1) * 512], po[:]
                                )

                        for m2 in range(TB):
                            nc.gpsimd.dma_scatter_add(
                                y_d[:],
                                out_sb[:, m2:m2 + 1, :],
                                idx_wr[
                                    :, m2 * (PD // 16):(m2 + 1) * (PD // 16)
                                ].bitcast(dt.int16),
                                PD,
                                PD,
                                D,
                            )

                if repeats > 1 and rep < repeats - 1:
                    # serialize repeats so the R-delta timing measures clean
                    # single-shot iterations (also avoids cross-repeat RMW races)
                    tc.strict_bb_all_engine_barrier()

    nc.compile()
    return nc


def _get_nc(repeats=1):
    key = f"nc{repeats}"
    if key not in _cache:
        _cache[key] = _build_nc(repeats)
    return _cache[key]


def timed_hw(in_maps, repeats=1, iters=6):
    """Min wall time of the sharded pjrt execute with device-resident
    inputs (fresh donated zero output buffers each call)."""
    import time

    import jax
    from jax.sharding import Mesh, PartitionSpec
    from jax.experimental.shard_map import shard_map
    import concourse.mybir as mybir
    from concourse import bass2jax

    nc = _get_nc(repeats)
    bass2jax.install_neuronx_cc_hook()

    partition_name = nc.partition_id_tensor.name if nc.partition_id_tensor else None
    in_names, out_names, out_avals, zero_shapes = [], [], [], []
    for alloc in nc.m.functions[0].allocations:
        if not isinstance(alloc, mybir.MemoryLocationSet):
            continue
        name = alloc.memorylocations[0].name
        if alloc.kind == "ExternalInput":
            if name != partition_name:
                in_names.append(name)
        elif alloc.kind == "ExternalOutput":
            out_names.append(name)
            shape = tuple(alloc.tensor_shape)
            dtype = mybir.dt.np(alloc.dtype)
            out_avals.append(jax.core.ShapedArray(shape, dtype))
            zero_shapes.append((shape, dtype))
    n_params = len(in_names)
    all_names = in_names + out_names
    if partition_name is not None:
        all_names = all_names + [partition_name]

    def _body(*args):
        operands = list(args)
        if partition_name is not None:
            operands.append(bass2jax.partition_id_tensor())
        outs = bass2jax._bass_exec_p.bind(
            *operands,
            out_avals=tuple(out_avals),
            in_names=tuple(all_names),
            out_names=tuple(out_names),
            lowering_input_output_aliases=(),
            sim_require_finite=True,
            sim_require_nnan=True,
            nc=nc,
        )
        return tuple(outs)

    devices = jax.devices()[:NCORES]
    mesh = Mesh(np.asarray(devices), ("core",))
    donate = tuple(range(n_params, n_params + len(out_names)))
    fn = jax.jit(
        shard_map(
            _body,
            mesh=mesh,
            in_specs=(PartitionSpec("core"),) * (n_params + len(out_names)),
            out_specs=(PartitionSpec("core"),) * len(out_names),
            check_rep=False,
        ),
        donate_argnums=donate,
        keep_unused=True,
    )
    sharding = jax.sharding.NamedSharding(mesh, PartitionSpec("core"))
    concat_in = [
        jax.device_put(
            np.concatenate([np.asarray(m[name]) for m in in_maps], axis=0), sharding
        )
        for name in in_names
    ]

    def fresh_zeros():
        return [
            jax.device_put(np.zeros((NCORES * s[0], *s[1:]), d), sharding)
            for (s, d) in zero_shapes
        ]

    times = []
    out = None
    for _ in range(iters):
        z = fresh_zeros()
        for zz in z:
            zz.block_until_ready()
        t0 = time.perf_counter()
        out = fn(*concat_in, *z)
        for o in out:
            o.block_until_ready()
        times.append(time.perf_counter() - t0)
    med = min(times)
    outs = [
        {
            name: np.asarray(out[i]).reshape(NCORES, *out_avals[i].shape)[c]
            for i, name in enumerate(out_names)
        }
        for c in range(NCORES)
    ]
    return med, times, outs


def make_in_maps(x, gate, W1, b1, W2, b2):
    x = np.asarray(x, dtype=np.float32)
    gate = np.asarray(gate, dtype=np.float32)
    W1 = np.asarray(W1, dtype=np.float32)
    b1 = np.asarray(b1, dtype=np.float32)
    W2 = np.asarray(W2, dtype=np.float32)
    b2 = np.asarray(b2, dtype=np.float32)
    ident = np.eye(PD, dtype=np.float32)
    NP0 = EG * NCH
    choff = (np.arange(NP0) // EG).astype(np.float32)[:, None] * CH
    smat = np.zeros((NP0, EG), dtype=np.float32)
    smat[np.arange(NP0), np.arange(NP0) % EG] = 1.0
    in_maps = []
    for c in range(NCORES):
        b = c // NG
        g = c % NG
        es = slice(g * EG, (g + 1) * EG)
        in_maps.append(
            {
                "xT": np.ascontiguousarray(x[b].T),
                "xrows": np.ascontiguousarray(x[b]),
                "gateg": np.ascontiguousarray(gate[:, es]),
                "w1g": np.ascontiguousarray(W1[es]),
                "b1g": np.ascontiguousarray(b1[es].reshape(1, -1)),
                "w2g": np.ascontiguousarray(W2[es]),
                "b2v": np.ascontiguousarray(b2[None, :]),
                "ident": ident,
                "choff": choff,
                "smat": smat,
            }
        )
    return in_maps


def run_spmd(in_maps, trace=False):
    from concourse.bass_utils import run_bass_kernel_spmd

    nc = _get_nc()
    return run_bass_kernel_spmd(nc, in_maps, list(range(NCORES)), trace=trace)


def combine(results):
    y = np.zeros((B, S, D), dtype=np.float32)
    for c in range(NCORES):
        y[c // NG] += results[c]["y"]
    return y


def kernel(x, gate, W1, b1, W2, b2, topk=K, **_unused):
    assert int(topk) == K, f"kernel hardcodes topk={K}, got {topk}"
    in_maps = make_in_maps(x, gate, W1, b1, W2, b2)
    # the first execute on a freshly-attached device occasionally fails with
    # NRT_EXEC_UNIT_UNRECOVERABLE and succeeds on retry
    last = None
    for _ in range(3):
        try:
            res = run_spmd(in_maps)
            return combine(res.results)
        except Exception as ex:  # noqa: BLE001
            last = ex
    raise last



# revision 10
# speedup vs baseline: 29.0667x; 29.0667x over previous
"""Expert-choice MoE FFN (B=2, S=2048, D=1024, E=16, k=256) on 8 trn2 cores.

Sharding: 8 cores = 2 batch shards x 4 expert-group shards (4 experts each).
Each core gets its batch's x (twice: transposed for the gate matmul, row-major
in DRAM for gathers) and its 4 experts' W1/W2/b1; b2 is replicated. The core
computes a partial y for its batch (scatter-add of its experts only); the host
sums the 4 group-partials per batch.

Per core:
  - logits^T (4, S) = gate_g^T @ x_b^T          (PE, exact fp32 for routing)
  - softmax stats over tokens                   (DVE reduce + ACT exp/accum)
  - top-256 per expert row in three stages:
      L0: per 256-token chunk (32 partitions) extract top-64 candidates
          (max8 + match_replace rounds; 64 >> expected per-chunk share 32)
      finish: flat top-256 of the 512 candidates per row (values only)
      re-find: recover token indices by max_index value-matching against the
          pristine chunked logits, then fold chunk offsets with a tiny matmul
  - per expert: dma_gather token rows, pre-scale by gate prob g (valid since
    g>0 commutes with relu), PE transpose, 2-layer FFN in fp32r with biases
    applied as K=1 fp32 matmul rows (b1*g, b2*g), dma_scatter_add into y.
"""

import os
import sys

sys.path.insert(0, "/opt/trn_rl_repo")

import numpy as np

B, S, D, E = 2, 2048, 1024, 16
NCORES = 8
NG = 4           # expert-group shards
EG = E // NG     # experts per core
K = 256          # top-k
PD = 128
KD = D // PD     # contraction chunks
TB = K // PD     # token blocks of 128
NEG = -3.0e38

NCH = 8          # token chunks per row for topk L0
CH = S // NCH    # 256 tokens per chunk
R0 = 56          # candidates kept per chunk (measured max share is 47)
NCAND = NCH * R0  # 512 candidates per row

_MM_FP32 = os.environ.get("KERNEL_MM_FP32", "0") == "1"

_cache = {}


def _build_nc(repeats=1):
    import concourse.bacc as bacc
    import concourse.mybir as mybir
    import concourse.tile as tile

    dt = mybir.dt
    Act = mybir.ActivationFunctionType
    RMM = dt.float32 if _MM_FP32 else dt.float32r

    nc = bacc.Bacc("TRN2", target_bir_lowering=False, debug=False, num_devices=NCORES)

    xT_d = nc.dram_tensor("xT", [D, S], dt.float32, kind="ExternalInput")
    xrow_d = nc.dram_tensor("xrows", [S, D], dt.float32, kind="ExternalInput")
    gate_d = nc.dram_tensor("gateg", [D, EG], dt.float32, kind="ExternalInput")
    w1_d = nc.dram_tensor("w1g", [EG, D, D], dt.float32, kind="ExternalInput")
    b1_d = nc.dram_tensor("b1g", [1, EG * D], dt.float32, kind="ExternalInput")
    w2_d = nc.dram_tensor("w2g", [EG, D, D], dt.float32, kind="ExternalInput")
    b2_d = nc.dram_tensor("b2v", [1, D], dt.float32, kind="ExternalInput")
    id_d = nc.dram_tensor("ident", [PD, PD], dt.float32, kind="ExternalInput")
    # chunked-topk partition layout: p = NCH*... -> p = 4*c + r (row r, chunk c)
    # choff[p] = CH * (p // EG); smat[p, r] = 1.0 if p % EG == r else 0
    choff_d = nc.dram_tensor("choff", [EG * NCH, 1], dt.float32, kind="ExternalInput")
    smat_d = nc.dram_tensor("smat", [EG * NCH, EG], dt.float32, kind="ExternalInput")
    y_d = nc.dram_tensor("y", [S, D], dt.float32, kind="ExternalOutput")
    # scratch DRAM for cross-partition reshapes
    ldr_d = nc.dram_tensor("ldr", [EG, S], dt.float32)
    cdr_d = nc.dram_tensor("cdr", [EG * NCH, R0], dt.float32)

    NP0 = EG * NCH  # 32 partitions used by the chunked topk stages

    with tile.TileContext(nc) as tc:
        with tc.tile_pool(name="persist", bufs=1) as pp:
            ident_sb = pp.tile([PD, PD], dt.float32, tag="ident")
            nc.sync.dma_start(ident_sb[:], id_d[:])
            b2_sb = pp.tile([1, D], dt.float32, tag="b2")
            nc.sync.dma_start(b2_sb[:], b2_d[:])
            b1_sb = pp.tile([1, EG * D], dt.float32, tag="b1")
            nc.sync.dma_start(b1_sb[:], b1_d[:])
            choff_sb = pp.tile([NP0, 1], dt.float32, tag="choff")
            nc.sync.dma_start(choff_sb[:], choff_d[:])
            smat_sb = pp.tile([NP0, EG], dt.float32, tag="smat")
            nc.sync.dma_start(smat_sb[:], smat_d[:])

            lchunk = pp.tile([NP0, CH], dt.float32, tag="lchunk")   # pristine
            lwork = pp.tile([NP0, CH], dt.float32, tag="lwork")     # destroyed
            cand = pp.tile([NP0, R0], dt.float32, tag="cand")
            candflat = pp.tile([EG, NCAND], dt.float32, tag="candflat")
            gv = pp.tile([EG, K], dt.float32, tag="gv")
            gvr = pp.tile([NP0, K], dt.float32, tag="gvr")
            posall = pp.tile([NP0, K], dt.uint16, tag="posall")
            posf = pp.tile([NP0, K], dt.float32, tag="posf")
            posmask = pp.tile([NP0, K], dt.float32, tag="posmask")
            post = pp.tile([NP0, K], dt.float32, tag="post")
            gi = pp.tile([EG, K], dt.uint16, tag="gi")
            sumexp = pp.tile([EG, 1], dt.float32, tag="sumexp")
            recip = pp.tile([EG, 1], dt.float32, tag="recip")
            gexp = pp.tile([EG, K], dt.float32, tag="gexp")
            g_all = pp.tile([EG, K], dt.float32, tag="g_all")

            # (no y zero-fill: both run_bass_kernel_spmd paths pre-zero
            # ExternalOutput buffers before the kernel runs)

            for rep in range(repeats):
              with (
                tc.tile_pool(name=f"w1p{rep}", bufs=3) as w1p,
                tc.tile_pool(name=f"w2p{rep}", bufs=2) as w2p,
                tc.tile_pool(name=f"xselp{rep}", bufs=3) as xsp,
                tc.tile_pool(name=f"xsTp{rep}", bufs=8) as xstp,
                tc.tile_pool(name=f"hp{rep}", bufs=8) as hp,
                tc.tile_pool(name=f"outp{rep}", bufs=2) as outp,
                tc.tile_pool(name=f"idxp{rep}", bufs=4) as idxp,
              ):
                # ---- Phase A: logits^T (EG, S) = gate_g^T @ x_b^T ----
                # k-outer so the first matmul fires after one 1MB x chunk
                with (
                    tc.tile_pool(name=f"xTp{rep}", bufs=2) as xTp,
                    tc.tile_pool(name=f"lsb{rep}", bufs=1) as lsbp,
                    tc.tile_pool(name=f"gatep{rep}", bufs=1) as gp,
                    tc.tile_pool(name=f"lpsum{rep}", bufs=4, space="PSUM") as lp,
                    tc.tile_pool(name=f"wups{rep}", bufs=1, space="PSUM") as wup,
                ):
                    # PE warmup: ~3.5us of back-to-back identity matmuls so
                    # the clock gate opens before the fp32 logits matmuls
                    # (fills the initial DMA wait with PE activity).
                    wtile = wup.tile([PD, PD], dt.float32, tag="warm")
                    for _ in range(32):
                        nc.tensor.matmul(
                            wtile[:], ident_sb[:], ident_sb[:],
                            start=True, stop=True,
                        )
                    gate_sb = gp.tile([PD, KD, EG], dt.float32, tag="gate")
                    nc.sync.dma_start(
                        gate_sb[:], gate_d[:].rearrange("(k p) e -> p k e", p=PD)
                    )
                    NL = S // 512
                    lps = [
                        lp.tile([EG, 512], dt.float32, tag="lps", name=f"lps{rep}_{n}")
                        for n in range(NL)
                    ]
                    for k in range(KD):
                        xt = xTp.tile([PD, S], dt.float32, tag="xT")
                        nc.sync.dma_start(xt[:], xT_d[k * PD:(k + 1) * PD, :])
                        for n in range(NL):
                            nc.tensor.matmul(
                                lps[n][:],
                                gate_sb[:, k, :],
                                xt[:, n * 512:(n + 1) * 512],
                                start=(k == 0),
                                stop=(k == KD - 1),
                            )
                    # stage logits to SBUF (DMA cannot read PSUM), then
                    # roundtrip via DRAM into the chunked layout
                    # (partition EG*c + r holds logits[r, CH*c:CH*(c+1)])
                    logits_sb = lsbp.tile(
                        [EG, S], dt.float32, tag="logsb", name=f"logsb{rep}"
                    )
                    for n in range(NL):
                        nc.vector.tensor_copy(
                            logits_sb[:, n * 512:(n + 1) * 512], lps[n][:]
                        )
                    nc.gpsimd.dma_start(ldr_d[:], logits_sb[:])
                nc.gpsimd.dma_start(
                    lchunk[:],
                    ldr_d[:].rearrange("r (c t) -> c r t", c=NCH),
                )
                nc.vector.tensor_copy(lwork[:], lchunk[:])

                # softmax stats in chunk layout (overlaps the L0 extraction).
                # Constant stability shift 0: logits are ~N(0,1), exp cannot
                # overflow fp32, and any constant cancels in the ratio.
                with tc.tile_pool(name=f"scratchp{rep}", bufs=1) as sp:
                    esc = sp.tile([NP0, CH], dt.float32, tag="esc")
                    acc32 = sp.tile([NP0, 1], dt.float32, tag="acc32")
                    nc.scalar.activation(
                        esc[:],
                        lchunk[:],
                        Act.Exp,
                        bias=0.0,
                        scale=1.0,
                        accum_out=acc32[:, 0:1],
                    )
                    with tc.tile_pool(
                        name=f"sepsum{rep}", bufs=1, space="PSUM"
                    ) as sep:
                        seps = sep.tile([EG, 1], dt.float32, tag="seps")
                        nc.tensor.matmul(
                            seps[:], smat_sb[:], acc32[:], start=True, stop=True
                        )
                        nc.vector.tensor_copy(sumexp[:], seps[:])
                nc.vector.reciprocal(recip[:], sumexp[:])

                # ---- Phase B: top-256 per row ----
                # L0: top-R0 of each chunk
                for r in range(R0 // 8):
                    cv = cand[:, 8 * r:8 * r + 8]
                    nc.vector.max(cv, lwork[:])
                    nc.vector.match_replace(lwork[:], cv, lwork[:], NEG)
                # merge candidates into one row per expert (via DRAM)
                nc.gpsimd.dma_start(cdr_d[:], cand[:])
                nc.gpsimd.dma_start(
                    candflat[:],
                    cdr_d[:].rearrange("(c r) j -> r c j", c=NCH),
                )
                # finish: top-K values of the candidates (sorted desc)
                for r in range(K // 8):
                    mv = gv[:, 8 * r:8 * r + 8]
                    nc.vector.max(mv, candflat[:])
                    nc.vector.match_replace(candflat[:], mv, candflat[:], NEG)
                # re-find token indices: replicate gv to each chunk block
                # (log-doubling), then value-match per chunk
                nc.gpsimd.dma_start(gvr[0:EG, :], gv[:])
                w = EG
                while w < EG * NCH:
                    nc.gpsimd.dma_start(gvr[w:2 * w, :], gvr[0:w, :])
                    w *= 2
                for r in range(K // 8):
                    nc.vector.max_index(
                        posall[:, 8 * r:8 * r + 8], gvr[:, 8 * r:8 * r + 8], lchunk[:]
                    )
                nc.vector.tensor_copy(posf[:], posall[:])
                nc.vector.tensor_single_scalar(
                    posmask[:], posf[:], 60000.0, mybir.AluOpType.is_lt
                )
                nc.vector.tensor_scalar_add(post[:], posf[:], choff_sb[:, 0:1])
                nc.vector.tensor_mul(post[:], post[:], posmask[:])
                with tc.tile_pool(name=f"gpsum{rep}", bufs=1, space="PSUM") as gpp:
                    gps = gpp.tile([EG, K], dt.float32, tag="gps")
                    nc.tensor.matmul(gps[:], smat_sb[:], post[:], start=True, stop=True)
                    nc.vector.tensor_copy(gi[:], gps[:])

                # ---- Phase C: gate probabilities of the selected tokens ----
                nc.scalar.activation(
                    gexp[:], gv[:], Act.Exp, bias=0.0, scale=1.0
                )
                nc.vector.tensor_scalar_mul(g_all[:], gexp[:], recip[:, 0:1])

                # ---- Phase D: per-expert gather -> FFN -> scatter-add ----
                with (
                    tc.tile_pool(name=f"pst{rep}", bufs=2, space="PSUM") as pst,
                    tc.tile_pool(name=f"ps1{rep}", bufs=3, space="PSUM") as ps1,
                    tc.tile_pool(name=f"ps2{rep}", bufs=2, space="PSUM") as ps2,
                ):
                    # W loads: no deps, so they prefetch during routing as
                    # pool slots allow (sync queue, behind the xT stream).
                    w1_tiles, w2_tiles = [], []
                    for e in range(EG):
                        wh = []
                        for half in range(2):
                            t = w1p.tile(
                                [PD, KD // 2, D], RMM, tag="w1",
                                name=f"w1_{rep}_{e}_{half}",
                            )
                            nc.sync.dma_start(
                                t[:],
                                w1_d[
                                    e, half * (D // 2):(half + 1) * (D // 2), :
                                ].bitcast(RMM).rearrange("(kk p) d -> p kk d", p=PD),
                            )
                            wh.append(t)
                        w1_tiles.append(wh)
                        wh = []
                        for half in range(2):
                            t = w2p.tile(
                                [PD, KD // 2, D], RMM, tag="w2",
                                name=f"w2_{rep}_{e}_{half}",
                            )
                            nc.sync.dma_start(
                                t[:],
                                w2_d[
                                    e, half * (D // 2):(half + 1) * (D // 2), :
                                ].bitcast(RMM).rearrange("(kk p) d -> p kk d", p=PD),
                            )
                            wh.append(t)
                        w2_tiles.append(wh)
                    # Index/gate prep + gathers for ALL experts up front, so
                    # the in-order Pool queue never parks a gather behind a
                    # scatter of an earlier expert.
                    idx_wrs, g_rows, g_cols, x_sels = [], [], [], []
                    for e in range(EG):
                        # The DMA gather/scatter unwrap idxs as [16, K//16]
                        # with list position i at [i%16, i//16]. Writing the
                        # row PLAIN into (16,16) permutes the list order by
                        # the 16x16 transpose pi(i) = 16*(i%16) + i//16 —
                        # fine as long as g uses the same permuted order.
                        # Replicate to all 8 groups by log-doubling.
                        idx_wr = idxp.tile(
                            [PD, K // 16], dt.uint16, tag="idxwr",
                            name=f"idxwr_{rep}_{e}", bufs=4,
                        )
                        nc.scalar.dma_start(idx_wr[0:16, :], gi[e:e + 1, :])
                        w = 16
                        while w < PD:
                            nc.scalar.dma_start(idx_wr[w:2 * w, :], idx_wr[0:w, :])
                            w *= 2
                        # g in permuted stream order (bias-row operand, base 0)
                        g_tmp0 = idxp.tile(
                            [1, K], dt.float32, tag="gtmp0",
                            name=f"gtmp0_{rep}_{e}", bufs=2,
                        )
                        nc.scalar.dma_start(g_tmp0[:], g_all[e:e + 1, :])
                        g_row = idxp.tile(
                            [1, K], dt.float32, tag="grow",
                            name=f"grow_{rep}_{e}", bufs=4,
                        )
                        nc.vector.tensor_copy(
                            g_row[:], g_tmp0[:].rearrange("a (s p) -> a p s", p=16)
                        )
                        # (128,2) per-partition scale: g_col[p,c]=g_row[128c+p]
                        g_colrow = idxp.tile(
                            [1, K], dt.float32, tag="gcolrow",
                            name=f"gcolrow_{rep}_{e}", bufs=2,
                        )
                        nc.vector.tensor_copy(
                            g_colrow[:].rearrange("a (p c) -> a p c", p=PD),
                            g_row[:].rearrange("a (c p) -> a p c", p=PD),
                        )
                        g_col = idxp.tile(
                            [PD, TB], dt.float32, tag="gcol",
                            name=f"gcol_{rep}_{e}", bufs=4,
                        )
                        nc.scalar.dma_start(g_col[:], g_colrow[:])

                        x_sel = xsp.tile(
                            [PD, TB, D], dt.float32, tag="xsel",
                            name=f"xsel_{rep}_{e}",
                        )
                        nc.gpsimd.dma_gather(
                            x_sel[:], xrow_d[:], idx_wr[:].bitcast(dt.int16), K, K, D
                        )
                        idx_wrs.append(idx_wr)
                        g_rows.append(g_row)
                        g_cols.append(g_col)
                        x_sels.append(x_sel)

                    for e in range(EG):
                        idx_wr = idx_wrs[e]
                        g_row = g_rows[e]
                        g_col = g_cols[e]
                        x_sel = x_sels[e]
                        w1_at = lambda k: w1_tiles[e][k // 4][:, k % 4, :]  # noqa: E731, B023
                        w2_at = lambda k: w2_tiles[e][k // 4][:, k % 4, :]  # noqa: E731, B023
                        for c in range(TB):
                            nc.scalar.activation(
                                x_sel[:, c, :],
                                x_sel[:, c, :],
                                Act.Copy,
                                bias=0.0,
                                scale=g_col[:, c:c + 1],
                            )
                        # transpose to (d, token); ACT copies round to fp32r
                        xsT = [
                            xstp.tile([PD, K], RMM, tag="xsT", name=f"xsT_{rep}_{e}_{j}")
                            for j in range(KD)
                        ]
                        for j in range(KD):
                            for c in range(TB):
                                tp = pst.tile([PD, PD], dt.float32, tag="trp")
                                nc.tensor.transpose(
                                    tp[:], x_sel[:, c, j * PD:(j + 1) * PD], ident_sb[:]
                                )
                                nc.vector.tensor_copy(
                                    xsT[j][:, c * PD:(c + 1) * PD], tp[:]
                                )

                        h_tiles = []
                        for m in range(KD):
                            ph = ps1.tile([PD, K], dt.float32, tag="ps1")
                            for k in range(KD):
                                nc.tensor.matmul(
                                    ph[:],
                                    w1_at(k)[:, m * PD:(m + 1) * PD],
                                    xsT[k][:],
                                    start=(k == 0),
                                    stop=False,
                                )
                            # + g * b1  (K=1 fp32 matmul row)
                            nc.tensor.matmul(
                                ph[:],
                                b1_sb[0:1, e * D + m * PD:e * D + (m + 1) * PD],
                                g_row[:],
                                start=False,
                                stop=True,
                            )
                            ht = hp.tile([PD, K], RMM, tag="h")
                            nc.scalar.activation(
                                ht[:], ph[:], Act.Relu, bias=0.0, scale=1.0
                            )
                            h_tiles.append(ht)

                        out_sb = outp.tile([PD, TB, D], dt.float32, tag="outsb")
                        for m2 in range(TB):
                            for n in range(2):
                                po = ps2.tile([PD, 512], dt.float32, tag="ps2")
                                for k in range(KD):
                                    nc.tensor.matmul(
                                        po[:],
                                        h_tiles[k][:, m2 * PD:(m2 + 1) * PD],
                                        w2_at(k)[:, n * 512:(n + 1) * 512],
                                        start=(k == 0),
                                        stop=False,
                                    )
                                # + g * b2  (K=1 fp32 matmul row)
                                nc.tensor.matmul(
                                    po[:],
                                    g_row[0:1, m2 * PD:(m2 + 1) * PD],
                                    b2_sb[:, n * 512:(n + 1) * 512],
                                    start=False,
                                    stop=True,
                                )
                                nc.vector.tensor_copy(
                                    out_sb[:, m2, n * 512:(n + 1) * 512], po[:]
                                )

                        for m2 in range(TB):
                            nc.gpsimd.dma_scatter_add(
                                y_d[:],
                                out_sb[:, m2:m2 + 1, :],
                                idx_wr[
                                    :, m2 * (PD // 16):(m2 + 1) * (PD // 16)
                                ].bitcast(dt.int16),
                                PD,
                                PD,
                                D,
                            )

                if repeats > 1 and rep < repeats - 1:
                    # serialize repeats so the R-delta timing measures clean
                    # single-shot iterations (also avoids cross-repeat RMW races)
                    tc.strict_bb_all_engine_barrier()

    nc.compile()
    return nc


def _get_nc(repeats=1):
    key = f"nc{repeats}"
    if key not in _cache:
        _cache[key] = _build_nc(repeats)
    return _cache[key]


def timed_hw(in_maps, repeats=1, iters=6):
    """Min wall time of the sharded pjrt execute with device-resident
    inputs (fresh donated zero output buffers each call)."""
    import time

    import jax
    from jax.sharding import Mesh, PartitionSpec
    from jax.experimental.shard_map import shard_map
    import concourse.mybir as mybir
    from concourse import bass2jax

    nc = _get_nc(repeats)
    bass2jax.install_neuronx_cc_hook()

    partition_name = nc.partition_id_tensor.name if nc.partition_id_tensor else None
    in_names, out_names, out_avals, zero_shapes = [], [], [], []
    for alloc in nc.m.functions[0].allocations:
        if not isinstance(alloc, mybir.MemoryLocationSet):
            continue
        name = alloc.memorylocations[0].name
        if alloc.kind == "ExternalInput":
            if name != partition_name:
                in_names.append(name)
        elif alloc.kind == "ExternalOutput":
            out_names.append(name)
            shape = tuple(alloc.tensor_shape)
            dtype = mybir.dt.np(alloc.dtype)
            out_avals.append(jax.core.ShapedArray(shape, dtype))
            zero_shapes.append((shape, dtype))
    n_params = len(in_names)
    all_names = in_names + out_names
    if partition_name is not None:
        all_names = all_names + [partition_name]

    def _body(*args):
        operands = list(args)
        if partition_name is not None:
            operands.append(bass2jax.partition_id_tensor())
        outs = bass2jax._bass_exec_p.bind(
            *operands,
            out_avals=tuple(out_avals),
            in_names=tuple(all_names),
            out_names=tuple(out_names),
            lowering_input_output_aliases=(),
            sim_require_finite=True,
            sim_require_nnan=True,
            nc=nc,
        )
        return tuple(outs)

    devices = jax.devices()[:NCORES]
    mesh = Mesh(np.asarray(devices), ("core",))
    donate = tuple(range(n_params, n_params + len(out_names)))
    fn = jax.jit(
        shard_map(
            _body,
            mesh=mesh,
            in_specs=(PartitionSpec("core"),) * (n_params + len(out_names)),
            out_specs=(PartitionSpec("core"),) * len(out_names),
            check_rep=False,
        ),
        donate_argnums=donate,
        keep_unused=True,
    )
    sharding = jax.sharding.NamedSharding(mesh, PartitionSpec("core"))
    concat_in = [
        jax.device_put(
            np.concatenate([np.asarray(m[name]) for m in in_maps], axis=0), sharding
        )
        for name in in_names
    ]

    def fresh_zeros():
        return [
            jax.device_put(np.zeros((NCORES * s[0], *s[1:]), d), sharding)
            for (s, d) in zero_shapes
        ]

    times = []
    out = None
    for _ in range(iters):
        z = fresh_zeros()
        for zz in z:
            zz.block_until_ready()
        t0 = time.perf_counter()
        out = fn(*concat_in, *z)
        for o in out:
            o.block_until_ready()
        times.append(time.perf_counter() - t0)
    med = min(times)
    outs = [
        {
            name: np.asarray(out[i]).reshape(NCORES, *out_avals[i].shape)[c]
            for i, name in enumerate(out_names)
        }
        for c in range(NCORES)
    ]
    return med, times, outs


def make_in_maps(x, gate, W1, b1, W2, b2):
    x = np.asarray(x, dtype=np.float32)
    gate = np.asarray(gate, dtype=np.float32)
    W1 = np.asarray(W1, dtype=np.float32)
    b1 = np.asarray(b1, dtype=np.float32)
    W2 = np.asarray(W2, dtype=np.float32)
    b2 = np.asarray(b2, dtype=np.float32)
    ident = np.eye(PD, dtype=np.float32)
    NP0 = EG * NCH
    choff = (np.arange(NP0) // EG).astype(np.float32)[:, None] * CH
    smat = np.zeros((NP0, EG), dtype=np.float32)
    smat[np.arange(NP0), np.arange(NP0) % EG] = 1.0
    in_maps = []
    for c in range(NCORES):
        b = c // NG
        g = c % NG
        es = slice(g * EG, (g + 1) * EG)
        in_maps.append(
            {
                "xT": np.ascontiguousarray(x[b].T),
                "xrows": np.ascontiguousarray(x[b]),
                "gateg": np.ascontiguousarray(gate[:, es]),
                "w1g": np.ascontiguousarray(W1[es]),
                "b1g": np.ascontiguousarray(b1[es].reshape(1, -1)),
                "w2g": np.ascontiguousarray(W2[es]),
                "b2v": np.ascontiguousarray(b2[None, :]),
                "ident": ident,
                "choff": choff,
                "smat": smat,
            }
        )
    return in_maps


def run_spmd(in_maps, trace=False):
    from concourse.bass_utils import run_bass_kernel_spmd

    nc = _get_nc()
    return run_bass_kernel_spmd(nc, in_maps, list(range(NCORES)), trace=trace)


def combine(results):
    y = np.zeros((B, S, D), dtype=np.float32)
    for c in range(NCORES):
        y[c // NG] += results[c]["y"]
    return y


def kernel(x, gate, W1, b1, W2, b2, topk=K, **_unused):
    assert int(topk) == K, f"kernel hardcodes topk={K}, got {topk}"
    in_maps = make_in_maps(x, gate, W1, b1, W2, b2)
    # the first execute on a freshly-attached device occasionally fails with
    # NRT_EXEC_UNIT_UNRECOVERABLE and succeeds on retry
    last = None
    for _ in range(3):
        try:
            res = run_spmd(in_maps)
            return combine(res.results)
        except Exception as ex:  # noqa: BLE001
            last = ex
    raise last

